# revision 34
# baseline (speedup 1.0000x reference)
"""DecoderAttention Bass/Tile kernel for TRN2, batch-parallel over 8 NeuronCores.

Each core handles one batch element:
  q = enc @ Qs + Qbs ; k = enc @ Ks + Kbs ; v = nrp @ Vs + Vbs   (per head)
  scores = q k^T / sqrt(64), causal mask, softmax
  out = (attn @ v) @ O + Ob

v2 layout strategy (all matmuls bf16 at full PE rate, fp32 PSUM accumulate):
  - enc/nrp transposed AND cast to bf16 on the host -> encT/nrpT [d, s] DMA'd
    straight into SBUF (no on-device PE transposes)
  - weights pre-packed host-side in bf16: qkw [pair, d, 256*8] holds the q|k
    column blocks per contraction chunk so one DMA per pair; Vs padded to
    [d, 16*65] with a ones column per head so attn@v also produces softmax
    row sums; O as [d, d]
  - scoresT [m, q] per head so exp output feeds attn@v without transposing
  - causal diagonal block: exp first, then GpSimd affine_select zeroes the
    q < m half of the diag block in the bf16 exp output (exp(x) is finite
    and the zeroed entries never reach the sums row)
  - exp folds the 1/sqrt(d_head) scale; no max subtraction (scores are O(1))
  - q/k projections for pair g+1 are interleaved into pair g's attention as
    PE filler (2 matmuls per chunk), so the attention middle phase is
    PE-dense and the scalar-engine exp stream hides underneath
  - softmax division deferred: one batched reciprocal at the end, broadcast
    across partitions with one-hot selector matmuls
PSUM budget: scores tag 2x2 banks + pz 1x2 + proj 2x1 = 8 banks.
"""

import numpy as np
import ml_dtypes

import concourse.bass as bass
import concourse.mybir as mybir
import concourse.tile as tile
from concourse import bacc
from concourse.bass_utils import run_bass_kernel_spmd
from concourse.masks import make_identity

N_HEADS, D_MODEL, D_HEAD = 16, 1024, 64
BATCH, SEQ = 8, 1024
P = 128
DCH = D_MODEL // P       # 8 contraction chunks
ST = SEQ // P            # 8 seq tiles
PAIRS = N_HEADS // 2     # 8 head pairs
VW = 65                  # v width per head incl. ones column
VTOT = N_HEADS * VW      # 1040
SCALE = 1.0 / np.sqrt(np.float32(D_HEAD))

F32 = mybir.dt.float32
F32R = mybir.dt.float32r
BF16 = mybir.dt.bfloat16
AF = mybir.ActivationFunctionType
NPBF16 = ml_dtypes.bfloat16

_CACHE = {}


def _bank_splits(q0):
    # PSUM-bank-aligned (n0, nw) column splits covering [q0, SEQ)
    if q0 < 512:
        return [(q0, 512 - q0), (512, 512)]
    return [(q0, SEQ - q0)]


def _bcast_row_ap(src, n):
    # DMA access pattern replicating a [n]-element DRAM row to 128 partitions
    return bass.AP(tensor=src.tensor, offset=src.offset, ap=[[0, P], [1, n]])


def _build_program():
    nc = bacc.Bacc("TRN2", target_bir_lowering=False, debug=False, num_devices=8)

    encT = nc.dram_tensor("encT", [D_MODEL, SEQ], BF16, kind="ExternalInput").ap()
    nrpT = nc.dram_tensor("nrpT", [D_MODEL, SEQ], BF16, kind="ExternalInput").ap()
    qkw = nc.dram_tensor("qkw", [PAIRS, P, 2 * D_MODEL], BF16, kind="ExternalInput").ap()
    vst = nc.dram_tensor("vst", [D_MODEL, VTOT], BF16, kind="ExternalInput").ap()
    ow = nc.dram_tensor("ow", [D_MODEL, D_MODEL], BF16, kind="ExternalInput").ap()
    qb = nc.dram_tensor("qb", [D_MODEL], F32, kind="ExternalInput").ap()
    kb = nc.dram_tensor("kb", [D_MODEL], F32, kind="ExternalInput").ap()
    vb = nc.dram_tensor("vb", [VTOT], F32, kind="ExternalInput").ap()
    ob = nc.dram_tensor("ob", [D_MODEL], F32, kind="ExternalInput").ap()
    out = nc.dram_tensor("out", [SEQ, D_MODEL], F32, kind="ExternalOutput").ap()
    sums_dram = nc.dram_tensor("sums_scratch", [N_HEADS, SEQ], F32).ap()
    rcp_dram = nc.dram_tensor("rcp_scratch", [P, P], F32R).ap()
    with tile.TileContext(nc) as tc:
        _kernel(tc, out, encT, nrpT, qkw, vst, ow, qb, kb, vb, ob,
                sums_dram, rcp_dram)
    nc.compile()
    return nc


def _kernel(tc, out, encT, nrpT, qkw, vst, ow, qb, kb, vb, ob, sums_dram, rcp_dram):
    nc = tc.nc

    smalls = tc.alloc_tile_pool(name="smalls", bufs=1)
    vb_bc = smalls.tile([P, VTOT], F32, tag="vb_bc", name="vb_bc")
    ob_bc = smalls.tile([P, D_MODEL], F32, tag="ob_bc", name="ob_bc")
    qb_col = smalls.tile([P, PAIRS], F32, tag="qb_col", name="qb_col")
    kb_col = smalls.tile([P, PAIRS], F32, tag="kb_col", name="kb_col")
    identf = smalls.tile([P, P], F32, tag="identf", name="identf")
    make_identity(nc, identf)
    ident = smalls.tile([P, P], F32R, tag="ident", name="ident")
    nc.vector.tensor_copy(ident, identf)

    # ---- SBUF-resident inputs/weights ----
    # SBUF pools are LIFO stacks per side: allocate so mid-kernel releases
    # (vw, then qkw; nrp_sb, then enc_sb on the right) pop in stack order.
    osb = tc.alloc_tile_pool(name="osb", bufs=1)
    owt = [osb.tile([P, D_MODEL], BF16, tag=f"ow{k}", name=f"owt{k}") for k in range(DCH)]
    va_pool = tc.alloc_tile_pool(name="va", bufs=1)
    va = [va_pool.tile([P, VTOT], BF16, tag=f"va{t}", name=f"va{t}") for t in range(ST)]
    zt_pool = tc.alloc_tile_pool(name="zt", bufs=1)
    zt = [zt_pool.tile([P, SEQ], BF16, tag=f"zt{k}", name=f"zt{k}") for k in range(DCH)]
    # fp32 staging for the k=0..3 half of the output projection (stage A),
    # computed as pair-7 attention filler; ob is folded in here
    partA_pool = tc.alloc_tile_pool(name="partA", bufs=1)
    partA = [partA_pool.tile([P, D_MODEL], F32, tag=f"pa{t}", name=f"pa{t}") for t in range(ST)]
    qtkt_pool = tc.alloc_tile_pool(name="qtkt", bufs=2)
    qkw_pool = tc.alloc_tile_pool(name="qkwsb", bufs=1)
    qkw_sb = [qkw_pool.tile([P, 2 * D_MODEL], BF16, tag=f"qkw{g}", name=f"qkw{g}") for g in range(PAIRS)]
    vw_pool = tc.alloc_tile_pool(name="vwsb", bufs=1)
    vw = [vw_pool.tile([P, VTOT], BF16, tag=f"vw{c}", name=f"vw{c}") for c in range(DCH)]
    enc_sb_pool = tc.alloc_tile_pool(name="encsb", bufs=1, side="right")
    enc_sb = [enc_sb_pool.tile([P, SEQ], BF16, tag=f"enc{c}", name=f"enc{c}") for c in range(DCH)]
    nrp_sb_pool = tc.alloc_tile_pool(name="nrpsb", bufs=1, side="right")
    nrp_sb = [nrp_sb_pool.tile([P, SEQ], BF16, tag=f"nrp{c}", name=f"nrp{c}") for c in range(DCH)]

    # all DMAs issued up front, first-needed first: v-proj inputs gate the
    # first PE work, so nrpT (sync queue) and vst (scalar queue) stream in
    # parallel from t=0; everything else follows behind on the same queues
    for c in range(DCH):
        for s0 in (0, 512):
            nc.sync.dma_start(out=nrp_sb[c][:, s0:s0 + 512],
                              in_=nrpT[c * P:(c + 1) * P, s0:s0 + 512])
    for c in range(DCH):
        for s0 in (0, 520):
            nc.scalar.dma_start(out=vw[c][:, s0:s0 + 520],
                                in_=vst[c * P:(c + 1) * P, s0:s0 + 520])
    nc.sync.dma_start(out=vb_bc, in_=_bcast_row_ap(vb, VTOT))
    nc.sync.dma_start(out=qb_col, in_=qb.rearrange("(g p) -> p g", p=P))
    nc.sync.dma_start(out=kb_col, in_=kb.rearrange("(g p) -> p g", p=P))
    nc.scalar.dma_start(out=qkw_sb[0], in_=qkw[0])
    for c in range(DCH):
        nc.sync.dma_start(out=enc_sb[c], in_=encT[c * P:(c + 1) * P, :])
    nc.sync.dma_start(out=ob_bc, in_=_bcast_row_ap(ob, D_MODEL))
    for g in range(1, PAIRS):
        nc.scalar.dma_start(out=qkw_sb[g], in_=qkw[g])
    for k in range(DCH):
        nc.scalar.dma_start(out=owt[k], in_=ow[k * P:(k + 1) * P, :])

    # persistent PSUM pool for interleaved q/k projection groups (2 banks)
    ppool = tc.alloc_tile_pool(name="pp", bufs=2, space="PSUM")

    def outproj_half(t, n0, ks, dest):
        # half-k accumulation of out[t-block, n0:n0+512] into a pp bank
        pp = ppool.tile([P, 512], F32, tag="pp", name="ppo")
        for j, k in enumerate(ks):
            nc.tensor.matmul(
                pp,
                zt[k][:, t * P:(t + 1) * P],
                owt[k][:, n0:n0 + 512],
                start=(j == 0), stop=(j == len(ks) - 1),
                skip_group_check=True,
            )
        dest(t, n0, pp)

    def outproj_ops_a():
        # 16 closures, one per (t, n0): k=0..5 partial + ob into partA (fp32)
        def dest(t, n0, pp):
            nc.vector.tensor_add(
                partA[t][:, n0:n0 + 512], pp, ob_bc[:, n0:n0 + 512])
        return [
            lambda t=t, n0=n0: outproj_half(t, n0, range(6), dest)
            for t in range(ST) for n0 in (0, 512)
        ]

    def alloc_qtkt():
        qt_t = qtkt_pool.tile([P, SEQ], BF16, tag="qt", name="qt")
        kt_t = qtkt_pool.tile([P, SEQ], BF16, tag="kt", name="kt")
        return qt_t, kt_t

    def proj_ops(g, qt_t, kt_t):
        # 32 closures: 4 accumulation groups (q/k x n0) of 8 c-chunk matmuls;
        # the last matmul of each group is followed by the bias-add evacuation
        ops = []
        for woff, bcol, dst in ((0, qb_col, qt_t), (P, kb_col, kt_t)):
            for n0 in (0, 512):
                state = {}

                def mm(c, woff=woff, bcol=bcol, dst=dst, n0=n0, state=state):
                    if c == 0:
                        state["pp"] = ppool.tile([P, 512], F32, tag="pp", name="pp")
                    nc.tensor.matmul(
                        state["pp"],
                        qkw_sb[g][:, c * 2 * P + woff:c * 2 * P + woff + P],
                        enc_sb[c][:, n0:n0 + 512],
                        start=(c == 0), stop=(c == DCH - 1),
                        skip_group_check=True,
                    )
                    if c == DCH - 1:
                        nc.vector.tensor_scalar_add(
                            out=dst[:, n0:n0 + 512],
                            in0=state["pp"],
                            scalar1=bcol[:, g:g + 1],
                        )

                for c in range(DCH):
                    ops.append(lambda c=c, mm=mm: mm(c))
        return ops

    # ---- phase 1: v projection -> va [m, 16*65] with ones columns ----
    # (first: its inputs stream in on both DMA queues, so PE starts ~1.5us in)
    with tc.tile_pool(name="pv", bufs=2, space="PSUM") as pv:
        for t in range(ST):
            pt = pv.tile([P, VTOT], F32, tag="pv", name="pvt")
            for c in range(DCH):
                for n0 in range(0, VTOT, 512):
                    nw = min(512, VTOT - n0)
                    nc.tensor.matmul(
                        pt[:, n0:n0 + nw],
                        nrp_sb[c][:, t * P:(t + 1) * P],
                        vw[c][:, n0:n0 + nw],
                        start=(c == 0), stop=(c == DCH - 1),
                        skip_group_check=True,
                    )
            # vb_bc has the per-(h,dh) bias, with 1.0 in each ones-column slot;
            # matmul wrote 0 there (vst ones-columns are zero), so add gives 1.0
            nc.vector.tensor_add(va[t], pt, vb_bc)

    nrp_sb_pool.release()
    vw_pool.release()

    # ---- phase 2: q/k projection for pair 0 (dedicated) ----
    qt0, kt0 = alloc_qtkt()
    for op in proj_ops(0, qt0, kt0):
        op()

    # ---- phase 3: attention, with next pair's q/k projection interleaved ----
    with tc.tile_pool(name="attn", bufs=3) as apool, \
         tc.tile_pool(name="rcp", bufs=1) as rpool, \
         tc.tile_pool(name="selp", bufs=1) as selp, \
         tc.tile_pool(name="ps_s", bufs=2, space="PSUM") as spool, \
         tc.tile_pool(name="ps_z", bufs=1, space="PSUM") as zpool:
        # sel8[g][j, p] = 1 where j == 2(g%4) + p // 64 — one-hot broadcast
        # selector against the half-local [8, SEQ] reciprocal tiles
        sel8 = []
        for b in range(PAIRS):
            self_f = selp.tile([PAIRS, P], F32, tag="self", name="self", bufs=2)
            nc.gpsimd.memset(self_f, 0.0)
            nc.gpsimd.affine_select(
                out=self_f.rearrange("j (a c) -> j a c", a=2),
                in_=self_f.rearrange("j (a c) -> j a c", a=2),
                compare_op=mybir.AluOpType.not_equal,
                fill=1.0, base=-2 * (b % 4),
                pattern=[[-1, 2], [0, D_HEAD]], channel_multiplier=1,
            )
            s_r = selp.tile([PAIRS, P], F32R, tag=f"sel{b}", name=f"sel{b}")
            nc.vector.tensor_copy(s_r, self_f)
            sel8.append(s_r)
        # sel16a[a][j, p] = 1 where j == 2a + p//64 — selects the (a, hh)
        # row of the PE-transposed pair-7 reciprocal tile rS [16, 128]
        sel16a = []
        for b in range(ST):
            s16f = selp.tile([N_HEADS, P], F32, tag="s16f", name="s16f", bufs=2)
            nc.gpsimd.memset(s16f, 0.0)
            nc.gpsimd.affine_select(
                out=s16f.rearrange("j (a c) -> j a c", a=2),
                in_=s16f.rearrange("j (a c) -> j a c", a=2),
                compare_op=mybir.AluOpType.not_equal,
                fill=1.0, base=-2 * b,
                pattern=[[-1, 2], [0, D_HEAD]], channel_multiplier=1,
            )
            s16r = selp.tile([N_HEADS, P], F32R, tag=f"s16_{b}", name=f"s16_{b}")
            nc.vector.tensor_copy(s16r, s16f)
            sel16a.append(s16r)

        r16a = rpool.tile([PAIRS, SEQ], F32R, tag="r16a", name="r16a")
        r16b = rpool.tile([PAIRS, SEQ], F32R, tag="r16b", name="r16b")
        srow_prev = None
        qt_cur, kt_cur = qt0, kt0
        qt_nxt = kt_nxt = None
        pending = []
        for h in range(N_HEADS):
            g, off = h // 2, (h % 2) * D_HEAD
            if h % 2 == 0 and g + 1 < PAIRS:
                qt_nxt, kt_nxt = alloc_qtkt()
                pending = proj_ops(g + 1, qt_nxt, kt_nxt)
            elif h == 14:
                # pair 7 has no next-pair projection; its filler is the
                # k=0..3 half of the output projection (zt[0..3] normalized
                # at h==9); all 16 groups run at the tail, covering the
                # reciprocal chain's DMA+DVE latency with PE work
                pending = outproj_ops_a()
            pz = zpool.tile([VW, SEQ], F32, tag="pz", name="pz")

            def av_mms(i, ae, pz=pz, h=h):
                q0 = i * P
                for n0, nw in _bank_splits(q0):
                    nc.tensor.matmul(
                        pz[:, n0:n0 + nw],
                        va[i][:, h * VW:(h + 1) * VW],
                        ae[:, n0:n0 + nw],
                        start=(i == 0), stop=(i == ST - 1),
                        skip_group_check=True,
                    )

            pend = None
            for i in range(ST):
                q0 = i * P
                ps = spool.tile([P, SEQ], F32, tag="ps", name="ps")
                ae = apool.tile([P, SEQ], BF16, tag="ae", name="ae")
                for n0, nw in _bank_splits(q0):
                    nc.tensor.matmul(
                        ps[:, n0:n0 + nw],
                        kt_cur[off:off + D_HEAD, q0:q0 + P],
                        qt_cur[off:off + D_HEAD, n0:n0 + nw],
                        start=True, stop=True,
                        skip_group_check=True,
                    )
                nc.scalar.activation(
                    out=ae[:, q0:SEQ], in_=ps[:, q0:SEQ],
                    func=AF.Exp, scale=float(SCALE),
                )
                # causal diag block: zero exp output where q < m
                nc.gpsimd.affine_select(
                    out=ae[:, q0:q0 + P], in_=ae[:, q0:q0 + P],
                    compare_op=mybir.AluOpType.is_ge,
                    fill=0.0, base=0,
                    pattern=[[1, P]], channel_multiplier=-1,
                )
                # attn@v delayed one chunk so exp latency hides behind PE work
                if pend is not None:
                    av_mms(*pend)
                if g == 7:
                    if pending:
                        pending.pop(0)()
                else:
                    for _ in range(2):
                        if pending:
                            pending.pop(0)()
                pend = (i, ae)
            av_mms(*pend)
            # stash unnormalized zT and the denominator row; frees PSUM slots
            nc.vector.tensor_copy(zt[g][off:off + D_HEAD, :], pz[0:D_HEAD, :])
            srow = rpool.tile([1, SEQ], F32, tag="srow", name="srow", bufs=2)
            nc.vector.tensor_copy(srow, pz[D_HEAD:VW, :])
            nc.sync.dma_start(out=sums_dram[h:h + 1, :], in_=srow)

            # normalization, split by head-halves so the first half's DRAM
            # roundtrips + reciprocal hide inside the back half of attention;
            # reciprocal runs on the [64-partition, 128] reshape so the
            # FD-bound divide parallelizes across partitions. SBUF APs can't
            # fold free dims into partitions, so the reshapes go via DRAM.
            def rcp_part(lo, n, r16h, out_r, tag):
                # heads [lo, lo+n) -> reciprocal rows [out_r, out_r+n) of r16h
                sh = rpool.tile([8 * n, P], F32, tag=f"s{tag}", name=f"s{tag}")
                nc.sync.dma_start(
                    out=sh,
                    in_=sums_dram[lo:lo + n].rearrange("h (a c) -> (h a) c", c=P))
                rh = rpool.tile([8 * n, P], F32R, tag=f"r{tag}", name=f"r{tag}")
                with nc.allow_low_precision(reason="softmax denominators are O(1)"):
                    nc.vector.reciprocal(out=rh, in_=sh)
                nc.sync.dma_start(out=rcp_dram[8 * lo:8 * lo + 8 * n, :], in_=rh)
                nc.sync.dma_start(
                    out=r16h[out_r:out_r + n, :],
                    in_=rcp_dram[8 * lo:8 * lo + 8 * n].rearrange(
                        "(h a) c -> h (a c)", h=n))

            def norm_pairs(ggs, r16h, rows=PAIRS):
                for gg in ggs:
                    pb = spool.tile([P, SEQ], F32, tag="ps", name="psb")
                    for n0 in (0, 512):
                        nc.tensor.matmul(
                            pb[:, n0:n0 + 512],
                            sel8[gg][0:rows, :], r16h[0:rows, n0:n0 + 512],
                            start=True, stop=True, skip_group_check=True,
                        )
                    nc.vector.tensor_mul(zt[gg], zt[gg], pb)

            if h == 7:
                rcp_part(0, 8, r16a, 0, "a")
            elif h == 9:
                norm_pairs(range(4), r16a, rows=8)
            elif h == 12:
                rcp_part(8, 4, r16b, 0, "b1")
            elif h == 13:
                norm_pairs((4, 5), r16b, rows=4)
            elif h == 14:
                rcp_part(12, 2, r16b, 4, "b2")
            elif h == 15:
                norm_pairs((6,), r16b, rows=6)
                # pair-7 normalization without DRAM hops: PE-transpose the
                # two sum rows into [c, (a h)] columns, fast 16-wide DVE
                # reciprocal, transpose back, then 8 one-hot broadcast
                # matmuls. All PE/DVE work -> no idle window before stage B.
                psT = ppool.tile([P, 512], F32, tag="pp", name="psT")
                for hh, sr in ((0, srow_prev), (1, srow)):
                    for a in range(ST):
                        nc.tensor.transpose(
                            psT[:, a * 2 + hh:a * 2 + hh + 1],
                            sr[:, a * P:(a + 1) * P],
                            identf[0:1, 0:1],
                        )
                rT = rpool.tile([P, 16], F32, tag="rT", name="rT")
                nc.vector.reciprocal(out=rT, in_=psT[:, 0:16])
                psR = ppool.tile([P, 512], F32, tag="pp", name="psR")
                nc.tensor.transpose(psR[0:16, 0:P], rT, identf)
                rS = rpool.tile([16, P], F32R, tag="rS", name="rS")
                nc.vector.tensor_copy(rS, psR[0:16, 0:P])
                pb7 = spool.tile([P, SEQ], F32, tag="ps", name="pb7")
                for a in range(ST):
                    nc.tensor.matmul(
                        pb7[:, a * P:(a + 1) * P], sel16a[a], rS,
                        start=True, stop=True, skip_group_check=True,
                    )
                nc.vector.tensor_mul(zt[7], zt[7], pb7)
                # stage B: k=6..7 output projection + stage-A combine + store
                for t in range(ST):
                    ot = apool.tile([P, D_MODEL], F32, tag="ot", name="ot", bufs=2)

                    def destb(t, n0, pp, ot=ot):
                        nc.vector.tensor_add(
                            ot[:, n0:n0 + 512], pp, partA[t][:, n0:n0 + 512])

                    for n0 in (0, 512):
                        outproj_half(t, n0, range(6, 8), destb)
                    nc.sync.dma_start(out=out[t * P:(t + 1) * P, :], in_=ot)

            srow_prev = srow
            if h % 2 == 1:
                for op in pending:
                    op()
                pending = []
                qt_cur, kt_cur = qt_nxt, kt_nxt

    ppool.release()
    enc_sb_pool.release()
    qkw_pool.release()

    for pool in (qtkt_pool, partA_pool, zt_pool, va_pool, osb, smalls):
        pool.release()


def _get_program():
    if "nc" not in _CACHE:
        _CACHE["nc"] = _build_program()
    return _CACHE["nc"]


def _pack_weights(Qs, Qbs, Ks, Kbs, Vs, Vbs, O, Ob):
    f = np.float32
    qst = np.transpose(np.asarray(Qs, f), (1, 0, 2)).reshape(D_MODEL, D_MODEL)
    kst = np.transpose(np.asarray(Ks, f), (1, 0, 2)).reshape(D_MODEL, D_MODEL)
    # qkw[g, p, c*256 + w]: w<128 -> Q col block, else K col block (pair g, chunk c)
    q4 = qst.reshape(DCH, P, PAIRS, P)
    k4 = kst.reshape(DCH, P, PAIRS, P)
    qk = np.concatenate([q4, k4], axis=-1)            # [c, p, g, 256]
    qkw = np.ascontiguousarray(
        qk.transpose(2, 1, 0, 3).reshape(PAIRS, P, 2 * D_MODEL).astype(NPBF16))
    vst = np.zeros((D_MODEL, VTOT), f)
    vbf = np.zeros((VTOT,), f)
    Vs = np.asarray(Vs, f)
    Vbs = np.asarray(Vbs, f)
    for h in range(N_HEADS):
        vst[:, h * VW:h * VW + D_HEAD] = Vs[h]
        vbf[h * VW:h * VW + D_HEAD] = Vbs[h]
        vbf[h * VW + D_HEAD] = 1.0
    vst = np.ascontiguousarray(vst.astype(NPBF16))
    owb = np.ascontiguousarray(np.asarray(O, f).reshape(D_MODEL, D_MODEL).astype(NPBF16))
    qbf = np.ascontiguousarray(np.asarray(Qbs, f).reshape(D_MODEL))
    kbf = np.ascontiguousarray(np.asarray(Kbs, f).reshape(D_MODEL))
    obf = np.ascontiguousarray(np.asarray(Ob, f).reshape(D_MODEL))
    return qkw, vst, owb, qbf, kbf, vbf, obf


def kernel(normalized_resid_pre, encoder_output, Qs, Qbs, Ks, Kbs, Vs, Vbs, O, Ob,
           _trace=False, _trace_kwargs=None):
    nc = _get_program()
    qkw, vst, owb, qbf, kbf, vbf, obf = _pack_weights(Qs, Qbs, Ks, Kbs, Vs, Vbs, O, Ob)
    enc = np.asarray(encoder_output, np.float32)
    nrp = np.asarray(normalized_resid_pre, np.float32)
    in_maps = []
    for b in range(BATCH):
        in_maps.append({
            "encT": np.ascontiguousarray(enc[b].T).astype(NPBF16),
            "nrpT": np.ascontiguousarray(nrp[b].T).astype(NPBF16),
            "qkw": qkw, "vst": vst, "ow": owb,
            "qb": qbf, "kb": kbf, "vb": vbf, "ob": obf,
        })
    res = run_bass_kernel_spmd(
        nc, in_maps, list(range(BATCH)),
        trace=_trace, **(_trace_kwargs or {}),
    )
    out = np.stack([res.results[b]["out"] for b in range(BATCH)], axis=0)
    if _trace:
        _CACHE["last_results"] = res
    return out
